# revision 1
# baseline (speedup 1.0000x reference)
"""Longformer-style windowed self-attention for TRN2, 8-core SPMD.

Sharding: 24 (batch, head) pairs -> 3 heads per core (core c gets batch c//4,
heads (c%4)*3 .. +3). Each core computes QKV projections for its head slice,
windowed attention (block 256, window +-256), and writes its [4096, 192]
output channel slice. Host gathers slices into the full [2, 4096, 768] output.

Matmuls run as float32r (full-rate fp32 path). Scores are computed
transposed ([keys, queries]) so softmax renormalization reduces over the
partition dim via a ones-column appended to V in the PV matmul; results are
PE-transposed back and divided by the row sums.
"""

import sys

for _p in ("/opt/trn_rl_repo", "/opt/pypackages"):
    if _p not in sys.path:
        sys.path.append(_p)

import numpy as np
from contextlib import ExitStack

import concourse.bass as bass
import concourse.bacc as bacc
import concourse.mybir as mybir
import concourse.tile as tile
from concourse.bass_utils import run_bass_kernel_spmd

F32 = mybir.dt.float32
R32 = mybir.dt.float32r
EXP = mybir.ActivationFunctionType.Exp

B, S, D = 2, 4096, 768
H, DH = 12, 64
W = 256                 # one-sided window / query block size
NB = S // W             # 16 query blocks
NKC = S // 128          # 32 key chunks of 128
HPC = 3                 # heads per core
N_CORES = 8


def r32(ap):
    return ap.bitcast(R32)


def _blocks_for_t(t):
    """Attention blocks whose inputs are complete after projection s-tile t."""
    if t == 0:
        return [0]
    if t == 7:
        return [13, 14, 15]
    return [2 * t - 1, 2 * t]


def build_program(has_bias, has_kmask):
    nc = bacc.Bacc("TRN2", target_bir_lowering=False, debug=False,
                   num_devices=N_CORES)
    hsT_d = nc.declare_dram_parameter("hsT", [D, S], R32, isOutput=False)
    wqk_d = nc.declare_dram_parameter("wqk", [D, 384], R32, isOutput=False)
    wv_d = nc.declare_dram_parameter("wv", [D, 256], R32, isOutput=False)
    msk_d = nc.declare_dram_parameter("masks", [128, 1024], R32, isOutput=False)
    von_d = nc.declare_dram_parameter("vones", [128, 96], R32, isOutput=False)
    idn_d = nc.declare_dram_parameter("ident", [128, 128], F32, isOutput=False)
    if has_bias:
        bqk_d = nc.declare_dram_parameter("bqk", [1, 384], R32, isOutput=False)
        bv_d = nc.declare_dram_parameter("bv", [1, 256], R32, isOutput=False)
    if has_kmask:
        kpad_d = nc.declare_dram_parameter("kpad", [128, NKC], F32, isOutput=False)
        qpad_d = nc.declare_dram_parameter("qpad", [128, NKC], F32, isOutput=False)
    out_d = nc.declare_dram_parameter("out", [S, HPC * DH], F32, isOutput=True)

    with tile.TileContext(nc) as tc, ExitStack() as ctx:
        const_p = ctx.enter_context(tc.tile_pool(name="const", bufs=1))
        hst_p = ctx.enter_context(tc.tile_pool(name="hst", bufs=3))
        qkt_p = ctx.enter_context(tc.tile_pool(name="qkt", bufs=1))
        vall_p = ctx.enter_context(tc.tile_pool(name="vall", bufs=1))
        pt_p = ctx.enter_context(tc.tile_pool(name="pt", bufs=5))
        wk_p = ctx.enter_context(tc.tile_pool(name="wk", bufs=5))
        ps_p = ctx.enter_context(tc.tile_pool(name="ps", bufs=2, space="PSUM"))
        sm_p = ctx.enter_context(tc.tile_pool(name="sm", bufs=2, space="PSUM"))

        # ---- constants / weights ----
        wqk_sb = const_p.tile([128, 6, 384], R32)
        wv_sb = const_p.tile([128, 6, 256], R32)
        nc.sync.dma_start(wqk_sb[:], wqk_d[:].rearrange("(c p) n -> p c n", p=128))
        nc.sync.dma_start(wv_sb[:], wv_d[:].rearrange("(c p) n -> p c n", p=128))
        msk_sb = const_p.tile([128, 1024], R32)
        nc.sync.dma_start(msk_sb[:], msk_d[:, :])
        idn_sb = const_p.tile([128, 128], F32)
        nc.sync.dma_start(idn_sb[:], idn_d[:, :])
        if has_bias:
            bqk_sb = const_p.tile([1, 384], R32)
            nc.sync.dma_start(bqk_sb[:], bqk_d[:, :])
            bv_sb = const_p.tile([1, 256], R32)
            nc.sync.dma_start(bv_sb[:], bv_d[:, :])
            ones_sb = const_p.tile([1, 512], R32)
            nc.vector.memset(ones_sb[:], 1.0)
        if has_kmask:
            kpad_sb = const_p.tile([128, NKC], F32)
            nc.sync.dma_start(kpad_sb[:], kpad_d[:, :])
            qpad_sb = const_p.tile([128, NKC], F32)
            nc.sync.dma_start(qpad_sb[:], qpad_d[:, :])

        # qT/kT for head pair (A,B): A on partitions 0:64, B on 64:128
        qt_ab = qkt_p.tile([128, S], R32)
        kt_ab = qkt_p.tile([128, S], R32)
        # solo head C gets base-0 tiles
        qt_c = qkt_p.tile([64, S], R32)
        kt_c = qkt_p.tile([64, S], R32)
        # v in [s, dh] layout: [128, key-chunk, (vA|1|vB|1|vC|1)]
        vall = vall_p.tile([128, NKC, 195], R32)
        ones_cols = vall[:].rearrange("p m (h x) -> p m h x", h=3)[:, :, :, 64:65]
        nc.sync.dma_start(
            ones_cols, von_d[:].rearrange("p (m h x) -> p m h x", h=3, x=1)
        )

        hst_tiles = {}

        def emit_proj_qk(t):
            s0 = 512 * t
            hst = hst_p.tile([128, 6, 512], R32)
            hst_tiles[t] = hst
            nc.sync.dma_start(
                hst[:],
                hsT_d[:].rearrange("(c p) s -> p c s", p=128)[:, :, s0 : s0 + 512],
            )
            # q/k projections: 3 pair-matmuls of M=128
            for j in range(3):
                pp = sm_p.tile([128, 512], F32, space="PSUM", tag="sm")
                for c in range(6):
                    nc.tensor.matmul(
                        pp[:],
                        (wqk_sb[:, c, 128 * j : 128 * j + 128]),
                        (hst[:, c, :]),
                        start=(c == 0),
                        stop=(c == 5 and not has_bias),
                    )
                if has_bias:
                    nc.tensor.matmul(
                        pp[:],
                        (bqk_sb[0:1, 128 * j : 128 * j + 128]),
                        (ones_sb[0:1, :]),
                        start=False,
                        stop=True,
                    )
                if j == 0:
                    nc.vector.tensor_copy(qt_ab[:, s0 : s0 + 512], pp[:])
                elif j == 1:
                    nc.vector.tensor_copy(kt_ab[:, s0 : s0 + 512], pp[:])
                else:
                    nc.vector.tensor_copy(qt_c[:, s0 : s0 + 512], pp[0:64, :])
                    scr = wk_p.tile([128, 512], R32, name="kc_scr")
                    nc.vector.tensor_copy(scr[64:128, :], pp[64:128, :])
                    nc.sync.dma_start(kt_c[:, s0 : s0 + 512], scr[64:128, :])
        def emit_proj_v(t, mms=(0, 1, 2, 3), done=True):
            s0 = 512 * t
            hst = hst_tiles.pop(t) if done else hst_tiles[t]
            # v projection: 4 s-subtiles of 128, packed two per PSUM tile
            for mm0 in mms[::2]:
                m = 4 * t + mm0
                pv = sm_p.tile([128, 512], F32, space="PSUM", tag="sm")
                for half, mm in enumerate((mm0, mm0 + 1)):
                    for c in range(6):
                        nc.tensor.matmul(
                            pv[:, 256 * half : 256 * half + 256],
                            (hst[:, c, 128 * mm : 128 * mm + 128]),
                            (wv_sb[:, c, :]),
                            start=(c == 0),
                            stop=(c == 5 and not has_bias),
                        )
                    if has_bias:
                        nc.tensor.matmul(
                            pv[:, 256 * half : 256 * half + 256],
                            (ones_sb[0:1, 0:128]),
                            (bv_sb[0:1, :]),
                            start=False,
                            stop=True,
                        )
                dst = vall[:, m : m + 2, :].rearrange(
                    "p m (h x) -> p m h x", h=3
                )[:, :, :, 0:64]
                src = pv[:].rearrange("p (m x) -> p m x", m=2)[
                    :, :, 0:192
                ].rearrange("p m (h x) -> p m h x", h=3)
                nc.vector.tensor_copy(dst, src)

        def emit_block(n):
            kept = [m for m in range(2 * n - 2, 2 * n + 4) if 0 <= m < NKC]
            j0 = kept[0] - (2 * n - 2)
            c0, c1 = 256 * j0, 256 * (j0 + len(kept))
            q0 = 256 * n

            # scores (transposed): [128 keys, 256 queries] per chunk.
            # Middle (unmasked) chunks first so exp/PV pipeline behind them.
            order = sorted(range(len(kept)), key=lambda i: abs(kept[i] - 2 * n - 0.5))

            def emit_scores(ps, kt, qt, tp):
                for i in order:
                    m = kept[i]
                    j = j0 + i
                    nc.tensor.matmul(
                        ps[:, 256 * j : 256 * j + 256],
                        (kt[:, 128 * m : 128 * m + 128]),
                        (qt[:, q0 : q0 + 256]),
                        start=True,
                        stop=True,
                        tile_position=tp,
                    )

            def emit_exp_mask(pt, ps):
                nc.scalar.activation(pt[:, c0:c1], ps[:, c0:c1], EXP)
                if n > 0:
                    nc.vector.tensor_mul(pt[:, 0:512], pt[:, 0:512], msk_sb[:, 0:512])
                if n < NB - 1:
                    nc.vector.tensor_mul(
                        pt[:, 1024:1536], pt[:, 1024:1536], msk_sb[:, 512:1024]
                    )
                if has_kmask:
                    for i, m in enumerate(kept):
                        j = j0 + i
                        nc.vector.tensor_scalar_mul(
                            pt[:, 256 * j : 256 * j + 256],
                            pt[:, 256 * j : 256 * j + 256],
                            kpad_sb[:, m : m + 1],
                        )

            ps_a = ps_p.tile([128, 1536], F32, space="PSUM", tag="ps")
            ps_b = ps_p.tile([128, 1536], F32, space="PSUM", tag="ps")
            for i in order:
                m = kept[i]
                j = j0 + i
                nc.tensor.matmul(
                    ps_a[:, 256 * j : 256 * j + 256],
                    (kt_ab[0:64, 128 * m : 128 * m + 128]),
                    (qt_ab[0:64, q0 : q0 + 256]),
                    start=True,
                    stop=True,
                    tile_position=(0, 0),
                )
                nc.tensor.matmul(
                    ps_b[:, 256 * j : 256 * j + 256],
                    (kt_ab[64:128, 128 * m : 128 * m + 128]),
                    (qt_ab[64:128, q0 : q0 + 256]),
                    start=True,
                    stop=True,
                    tile_position=(64, 0),
                )
            pt_a = pt_p.tile([128, 1536], R32, tag="pt")
            pt_b = pt_p.tile([128, 1536], R32, tag="pt")
            emit_exp_mask(pt_a, ps_a)
            emit_exp_mask(pt_b, ps_b)

            ps_c = ps_p.tile([128, 1536], F32, space="PSUM", tag="ps")
            emit_scores(ps_c, kt_c, qt_c, (0, 0))
            pt_c = pt_p.tile([128, 1536], R32, tag="pt")
            emit_exp_mask(pt_c, ps_c)

            # PV: outT_u[65, 256] per head; col 64 of lhsT is the ones column
            outp = sm_p.tile([128, 512], F32, space="PSUM", tag="sm")
            for x, pt in ((0, pt_a), (1, pt_b)):
                for oi, i in enumerate(order):
                    m = kept[i]
                    j = j0 + i
                    nc.tensor.matmul(
                        outp[0:65, 256 * x : 256 * x + 256],
                        (vall[:, m, 65 * x : 65 * x + 65]),
                        (pt[:, 256 * j : 256 * j + 256]),
                        start=(oi == 0),
                        stop=(oi == len(kept) - 1),
                    )
            ot_ab = wk_p.tile([65, 512], F32, name="ot_ab")
            nc.vector.tensor_copy(ot_ab[:], outp[0:65, :])

            outp_c = sm_p.tile([128, 512], F32, space="PSUM", tag="sm")
            for oi, i in enumerate(order):
                m = kept[i]
                j = j0 + i
                nc.tensor.matmul(
                    outp_c[0:65, 0:256],
                    (vall[:, m, 130:195]),
                    (pt_c[:, 256 * j : 256 * j + 256]),
                    start=(oi == 0),
                    stop=(oi == len(kept) - 1),
                )

            # transpose [65, 128] -> [128, 65]; col 64 holds the softmax denom
            trp = sm_p.tile([128, 512], F32, space="PSUM", tag="sm")
            for x in range(2):
                for g in range(2):
                    i4 = 2 * x + g
                    nc.tensor.transpose(
                        trp[:, 65 * i4 : 65 * i4 + 65],
                        ot_ab[0:65, 256 * x + 128 * g : 256 * x + 128 * g + 128],
                        idn_sb[0:65, 0:65],
                    )
            ot_c = wk_p.tile([65, 256], F32, name="ot_c")
            nc.vector.tensor_copy(ot_c[:], outp_c[0:65, 0:256])

            dcol = trp[:, 0:260].rearrange("p (i x) -> p i x", x=65)[:, :, 64:65]
            rec = wk_p.tile([128, 4], F32, name="rec")
            nc.vector.reciprocal(rec[:].rearrange("p (i x) -> p i x", x=1), dcol)
            for g in range(2):
                osb = wk_p.tile([128, 128], F32, name="osb")
                for x in range(2):
                    i4 = 2 * x + g
                    nc.vector.tensor_scalar_mul(
                        osb[:, 64 * x : 64 * x + 64],
                        trp[:, 65 * i4 : 65 * i4 + 64],
                        rec[:, i4 : i4 + 1],
                    )
                if has_kmask:
                    nc.vector.tensor_scalar_mul(
                        osb[:], osb[:], qpad_sb[:, 2 * n + g : 2 * n + g + 1]
                    )
                nc.sync.dma_start(
                    out_d[q0 + 128 * g : q0 + 128 * g + 128, 0:128], osb[:]
                )

            trp_c = sm_p.tile([128, 512], F32, space="PSUM", tag="sm")
            for g in range(2):
                nc.tensor.transpose(
                    trp_c[:, 65 * g : 65 * g + 65],
                    ot_c[0:65, 128 * g : 128 * g + 128],
                    idn_sb[0:65, 0:65],
                )
            dcol_c = trp_c[:, 0:130].rearrange("p (i x) -> p i x", x=65)[:, :, 64:65]
            rec_c = wk_p.tile([128, 2], F32, name="rec_c")
            nc.vector.reciprocal(rec_c[:].rearrange("p (i x) -> p i x", x=1), dcol_c)
            for g in range(2):
                osb_c = wk_p.tile([128, 64], F32, name="osb_c")
                nc.vector.tensor_scalar_mul(
                    osb_c[:], trp_c[:, 65 * g : 65 * g + 64], rec_c[:, g : g + 1]
                )
                if has_kmask:
                    nc.vector.tensor_scalar_mul(
                        osb_c[:], osb_c[:], qpad_sb[:, 2 * n + g : 2 * n + g + 1]
                    )
                nc.sync.dma_start(
                    out_d[q0 + 128 * g : q0 + 128 * g + 128, 128:192], osb_c[:]
                )

        # Interleave: a projection s-tile between attention blocks keeps PE
        # busy while the previous block's exp/mask/epilogue chains drain.
        def emit_proj(t):
            emit_proj_qk(t)
            emit_proj_v(t)

        emit_proj(0)
        emit_proj(1)
        emit_block(0)
        nb_next = 1
        for t in range(2, 8):
            emit_proj(t)
            emit_block(nb_next)
            emit_block(nb_next + 1)
            nb_next += 2
        for n in range(nb_next, NB):
            emit_block(n)

    nc.compile()
    return nc


_prog_cache = {}


def _get_program(has_bias, has_kmask):
    key = (has_bias, has_kmask)
    if key not in _prog_cache:
        _prog_cache[key] = build_program(has_bias, has_kmask)
    return _prog_cache[key]


def _band_masks():
    """Multiplicative band masks for window chunks 0,1,4,5: [128, 4*256]."""
    r = np.arange(128)[:, None]
    q = np.arange(256)[None, :]
    m0 = (q <= r).astype(np.float32)
    m1 = (q <= r + 128).astype(np.float32)
    m4 = (r <= q).astype(np.float32)
    m5 = (r + 128 <= q).astype(np.float32)
    return np.concatenate([m0, m1, m4, m5], axis=1)


def kernel(hidden_states, attention_mask, Wq, bq, Wk, bk, Wv, bv, _res=[None]):
    hidden_states = np.asarray(hidden_states, np.float32)
    attention_mask = np.asarray(attention_mask, np.float32)
    Wq, Wk, Wv = (np.asarray(w, np.float32) for w in (Wq, Wk, Wv))
    bq, bk, bv = (np.asarray(b_, np.float32) for b_ in (bq, bk, bv))

    scale = 1.0 / np.sqrt(DH)
    has_bias = bool(np.any(bq) or np.any(bk) or np.any(bv))
    has_kmask = bool(np.any(attention_mask < 0))

    hsT = [np.ascontiguousarray(hidden_states[b].T) for b in range(B)]
    masks = _band_masks()
    ident = np.eye(128, dtype=np.float32)
    masked = attention_mask < 0  # [B, S]

    in_maps = []
    for core in range(N_CORES):
        b, h0 = core // 4, (core % 4) * HPC
        sl = slice(h0 * DH, (h0 + HPC) * DH)
        wq = Wq[:, sl] * scale
        wk = Wk[:, sl]
        wqk = np.concatenate(
            [wq[:, 0:128], wk[:, 0:128], wq[:, 128:192], wk[:, 128:192]], axis=1
        )
        wv = np.zeros((D, 256), np.float32)
        wv[:, 0:192] = Wv[:, sl]
        m = {
            "hsT": hsT[b],
            "wqk": np.ascontiguousarray(wqk),
            "wv": wv,
            "masks": masks,
            "vones": np.ones((128, 96), np.float32),
            "ident": ident,
        }
        if has_bias:
            bq_s = bq[sl] * scale
            bk_s = bk[sl]
            m["bqk"] = np.concatenate(
                [bq_s[0:128], bk_s[0:128], bq_s[128:192], bk_s[128:192]]
            ).reshape(1, 384).astype(np.float32)
            bvp = np.zeros((1, 256), np.float32)
            bvp[0, 0:192] = bv[sl]
            m["bv"] = bvp
        if has_kmask:
            keep = (~masked[b]).astype(np.float32).reshape(NKC, 128).T
            m["kpad"] = np.ascontiguousarray(keep)
            m["qpad"] = np.ascontiguousarray(keep)
        in_maps.append(m)

    nc = _get_program(has_bias, has_kmask)
    res = run_bass_kernel_spmd(nc, in_maps, list(range(N_CORES)))
    _res[0] = res

    out = np.empty((B, S, D), np.float32)
    for core in range(N_CORES):
        b, h0 = core // 4, (core % 4) * HPC
        out[b, :, h0 * DH : (h0 + HPC) * DH] = res.results[core]["out"]
    return out



# revision 4
# speedup vs baseline: 1.4437x; 1.4437x over previous
"""Longformer-style windowed self-attention for TRN2, 8-core SPMD.

Sharding: 24 (batch, head) pairs -> 3 heads per core (core c gets batch c//4,
heads (c%4)*3 .. +3). Each core computes QKV projections for its head slice,
windowed attention (block 256, window +-256), and writes its [4096, 192]
output channel slice. Host gathers slices into the full [2, 4096, 768] output.

All matmul inputs are bf16 (inputs/weights converted on host). Scores are
computed transposed ([keys, queries]); probs (exp'd scores) become the
stationary operand of the PV matmul, which therefore produces output directly
in [queries, head_dim] layout with a ones-column carrying the softmax
denominator - no PE transposes needed. Band-mask multiplies run on GpSimd,
exp on the scalar engine, PSUM evacuation + normalize scaling on DVE.
"""

import sys

for _p in ("/opt/trn_rl_repo", "/opt/pypackages"):
    if _p not in sys.path:
        sys.path.append(_p)

import numpy as np
import ml_dtypes
from contextlib import ExitStack

import concourse.bass as bass
import concourse.bacc as bacc
import concourse.mybir as mybir
import concourse.tile as tile
from concourse.bass_utils import run_bass_kernel_spmd

F32 = mybir.dt.float32
BF16 = mybir.dt.bfloat16
EXP = mybir.ActivationFunctionType.Exp
MUL = mybir.AluOpType.mult

B, S, D = 2, 4096, 768
H, DH = 12, 64
W = 256                 # one-sided window / query block size
NB = S // W             # 16 query blocks
NKC = S // 128          # 32 key chunks of 128
HPC = 3                 # heads per core
N_CORES = 8


def block_layout(n):
    """Score-PSUM column layout for query block n.

    Returns (pieces, maskop, ncols). pieces = [(m, qlo, qhi, col)]: key chunk
    m's scores for local queries [qlo, qhi) live at psum cols [col, col+qhi-qlo).
    maskop = (dst_col, width, src_col) multiplies pt[:, dst:dst+width] by
    msk[:, src:src+width] (msk = [L|L|U|U]). 256-wide pieces sit at byte
    offsets that never straddle a 2KB PSUM bank.
    """
    if n == 0:
        pieces = [(0, 0, 256, 0), (1, 0, 256, 256),
                  (3, 128, 256, 512), (2, 0, 256, 640)]
        maskop = (512, 256, 256)  # [mR2 | mR1 tri] *= [U|U]
        ncols = 896
    elif n == NB - 1:
        m0 = 2 * n
        pieces = [(m0, 0, 256, 0), (m0 - 1, 0, 256, 256),
                  (m0 - 2, 0, 128, 512), (m0 + 1, 0, 256, 640)]
        maskop = (384, 256, 0)    # [mL1 tri | mL2] *= [L|L]
        ncols = 896
    else:
        pieces = [(2 * n - 1, 0, 256, 0), (2 * n - 2, 0, 128, 256),
                  (2 * n + 3, 128, 256, 384), (2 * n + 2, 0, 256, 512),
                  (2 * n, 0, 256, 768), (2 * n + 1, 0, 256, 1024)]
        maskop = (128, 512, 0)    # [mL1 tri | mL2 | mR2 | mR1 tri] *= [L|L|U|U]
        ncols = 1280
    return pieces, maskop, ncols


def pv_chunks(pieces, half):
    """(m, pt_col) for key chunks fully covering query half [128h, 128h+128)."""
    q0, q1 = 128 * half, 128 * half + 128
    return [(m, col + q0 - qlo) for (m, qlo, qhi, col) in pieces
            if qlo <= q0 and q1 <= qhi]


def build_program(has_bias, has_kmask):
    nc = bacc.Bacc("TRN2", target_bir_lowering=False, debug=False,
                   num_devices=N_CORES)
    hsT_d = nc.declare_dram_parameter("hsT", [D, S], BF16, isOutput=False)
    w_d = nc.declare_dram_parameter("wqkv", [D, 576], BF16, isOutput=False)
    msk_d = nc.declare_dram_parameter("masks", [128, 512], BF16, isOutput=False)
    if has_bias:
        bqkv_d = nc.declare_dram_parameter("bqkv", [1, 576], BF16, isOutput=False)
    if has_kmask:
        kpad_d = nc.declare_dram_parameter("kpad", [128, NKC], F32, isOutput=False)
        qpad_d = nc.declare_dram_parameter("qpad", [128, NKC], F32, isOutput=False)
    out_d = nc.declare_dram_parameter("out", [S, HPC * DH], F32, isOutput=True)

    with tile.TileContext(nc) as tc, ExitStack() as ctx:
        const_p = ctx.enter_context(tc.tile_pool(name="const", bufs=1))
        hst_p = ctx.enter_context(tc.tile_pool(name="hst", bufs=3))
        qkt_p = ctx.enter_context(tc.tile_pool(name="qkt", bufs=1))
        vall_p = ctx.enter_context(tc.tile_pool(name="vall", bufs=1))
        pt_p = ctx.enter_context(tc.tile_pool(name="pt", bufs=4))
        wk_p = ctx.enter_context(tc.tile_pool(name="wk", bufs=4))
        ps_p = ctx.enter_context(tc.tile_pool(name="ps", bufs=2, space="PSUM"))
        sm_p = ctx.enter_context(tc.tile_pool(name="sm", bufs=2, space="PSUM"))

        # ---- constants / weights ----
        wsb = const_p.tile([128, 6, 576], BF16)
        w_r = w_d[:].rearrange("(c p) n -> p c n", p=128)
        nc.sync.dma_start(wsb[:, :, 0:384], w_r[:, :, 0:384])

        hst_tiles = {}

        def dma_hst(t):
            hst = hst_p.tile([128, 6, 512], BF16)
            hst_tiles[t] = hst
            src = hsT_d[:].rearrange("(c p) s -> p c s", p=128)[
                :, :, 512 * t : 512 * t + 512
            ]
            if t == 0:  # split so the first projection group starts sooner
                nc.sync.dma_start(hst[:, 0:3, :], src[:, 0:3, :])
                nc.sync.dma_start(hst[:, 3:6, :], src[:, 3:6, :])
            else:
                nc.sync.dma_start(hst[:], src)

        dma_hst(0)
        nc.sync.dma_start(wsb[:, :, 384:576], w_r[:, :, 384:576])
        msk_sb = const_p.tile([128, 512], BF16)
        nc.sync.dma_start(msk_sb[:], msk_d[:, :])
        dma_hst(1)
        if has_bias:
            bqkv_sb = const_p.tile([1, 576], BF16)
            nc.sync.dma_start(bqkv_sb[:], bqkv_d[:, :])
            ones_sb = const_p.tile([1, 512], BF16)
            nc.vector.memset(ones_sb[:], 1.0)
        if has_kmask:
            kpad_sb = const_p.tile([128, NKC], F32)
            nc.sync.dma_start(kpad_sb[:], kpad_d[:, :])
            qpad_sb = const_p.tile([128, NKC], F32)
            nc.sync.dma_start(qpad_sb[:], qpad_d[:, :])

        # qT/kT for head pair (A,B): A on partitions 0:64, B on 64:128
        qt_ab = qkt_p.tile([128, S], BF16)
        kt_ab = qkt_p.tile([128, S], BF16)
        # solo head C: base-0 tiles
        qt_c = qkt_p.tile([64, S], BF16)
        kt_c = qkt_p.tile([64, S], BF16)
        # v in [key, dh] layout: [128, key-chunk, (vA|1|vB|1|vC|1)]
        vall = vall_p.tile([128, NKC, 195], BF16)
        ones_cols = vall[:].rearrange("p m (h x) -> p m h x", h=3)[:, :, :, 64:65]
        nc.vector.memset(ones_cols, 1.0)

        def emit_proj_qk(t):
            s0 = 512 * t
            if t + 1 < 8:
                dma_hst(t + 1)
            hst = hst_tiles[t]
            for j in range(3):
                pp = sm_p.tile([128, 512], F32, space="PSUM", tag="sm")
                for c in range(6):
                    nc.tensor.matmul(
                        pp[:],
                        (wsb[:, c, 128 * j : 128 * j + 128]),
                        (hst[:, c, :]),
                        start=(c == 0),
                        stop=(c == 5 and not has_bias),
                    )
                if has_bias:
                    nc.tensor.matmul(
                        pp[:],
                        (bqkv_sb[0:1, 128 * j : 128 * j + 128]),
                        (ones_sb[0:1, :]),
                        start=False,
                        stop=True,
                    )
                if j == 0:
                    nc.vector.tensor_copy(qt_ab[:, s0 : s0 + 512], pp[:])
                elif j == 1:
                    nc.vector.tensor_copy(kt_ab[:, s0 : s0 + 512], pp[:])
                else:
                    nc.vector.tensor_copy(qt_c[:, s0 : s0 + 512], pp[0:64, :])
                    kcs = wk_p.tile([128, 512], BF16, name="kcs")
                    nc.vector.tensor_copy(kcs[64:128, :], pp[64:128, :])
                    nc.sync.dma_start(kt_c[:, s0 : s0 + 512], kcs[64:128, :])

        def emit_proj_v(t):
            hst = hst_tiles.pop(t)
            for mm0 in (0, 2):
                m = 4 * t + mm0
                pv = sm_p.tile([128, 512], F32, space="PSUM", tag="sm")
                for half, mm in enumerate((mm0, mm0 + 1)):
                    for c in range(6):
                        nc.tensor.matmul(
                            pv[:, 256 * half : 256 * half + 192],
                            (hst[:, c, 128 * mm : 128 * mm + 128]),
                            (wsb[:, c, 384:576]),
                            start=(c == 0),
                            stop=(c == 5 and not has_bias),
                        )
                    if has_bias:
                        nc.tensor.matmul(
                            pv[:, 256 * half : 256 * half + 192],
                            (ones_sb[0:1, 0:128]),
                            (bqkv_sb[0:1, 384:576]),
                            start=False,
                            stop=True,
                        )
                dst = vall[:, m : m + 2, :].rearrange(
                    "p m (h x) -> p m h x", h=3
                )[:, :, :, 0:64]
                src = pv[:].rearrange("p (m x) -> p m x", m=2)[
                    :, :, 0:192
                ].rearrange("p m (h x) -> p m h x", h=3)
                nc.vector.tensor_copy(dst, src)

        HEADS = (
            (lambda: kt_ab[0:64, :], lambda: qt_ab[0:64, :]),
            (lambda: kt_ab[64:128, :], lambda: qt_ab[64:128, :]),
            (lambda: kt_c[:, :], lambda: qt_c[:, :]),
        )

        def emit_block(n):
            pieces, maskop, ncols = block_layout(n)
            q0 = 256 * n
            pts = []
            for h, (ktf, qtf) in enumerate(HEADS):
                kt, qt = ktf(), qtf()
                ps = ps_p.tile([128, 1280], F32, space="PSUM", tag="ps")
                for m, qlo, qhi, col in pieces:
                    nc.tensor.matmul(
                        ps[:, col : col + qhi - qlo],
                        (kt[:, 128 * m : 128 * m + 128]),
                        (qt[:, q0 + qlo : q0 + qhi]),
                        start=True,
                        stop=True,
                    )
                pt = pt_p.tile([128, 1280], BF16, tag="pt")
                pts.append(pt)
                nc.scalar.activation(pt[:, 0:ncols], ps[:, 0:ncols], EXP)
                dcol, width, scol = maskop
                nc.gpsimd.tensor_mul(
                    pt[:, dcol : dcol + width],
                    pt[:, dcol : dcol + width],
                    msk_sb[:, scol : scol + width],
                )
                if has_kmask:
                    for m, qlo, qhi, col in pieces:
                        nc.vector.tensor_scalar_mul(
                            pt[:, col : col + qhi - qlo],
                            pt[:, col : col + qhi - qlo],
                            kpad_sb[:, m : m + 1],
                        )

            # PV: out[q, dh] = pt(chunk).T @ [v|1]; col 64 of each head's rhs
            # slice is the ones column carrying the softmax denominator.
            outp = sm_p.tile([128, 512], F32, space="PSUM", tag="sm")
            for h, pt in enumerate(pts):
                for half in (0, 1):
                    chunks = pv_chunks(pieces, half)
                    for ci, (m, pcol) in enumerate(chunks):
                        nc.tensor.matmul(
                            outp[:, 256 * half + 65 * h : 256 * half + 65 * h + 65],
                            (pt[:, pcol : pcol + 128]),
                            (vall[:, m, 65 * h : 65 * h + 65]),
                            start=(ci == 0),
                            stop=(ci == len(chunks) - 1),
                        )

            rec = wk_p.tile([128, 8], F32, name="rec")
            osb = wk_p.tile([128, 2, 192], F32, name="osb")
            for half in (0, 1):
                dcols = outp[:, 256 * half : 256 * half + 195].rearrange(
                    "p (i x) -> p i x", x=65
                )[:, :, 64:65]
                nc.vector.reciprocal(
                    rec[:, 4 * half : 4 * half + 3].rearrange(
                        "p (i x) -> p i x", x=1
                    ),
                    dcols,
                )
                for h in range(3):
                    nc.vector.tensor_scalar_mul(
                        osb[:, half, 64 * h : 64 * h + 64],
                        outp[:, 256 * half + 65 * h : 256 * half + 65 * h + 64],
                        rec[:, 4 * half + h : 4 * half + h + 1],
                    )
                if has_kmask:
                    nc.vector.tensor_scalar_mul(
                        osb[:, half, :],
                        osb[:, half, :],
                        qpad_sb[:, 2 * n + half : 2 * n + half + 1],
                    )
            nc.sync.dma_start(
                out_d[q0 : q0 + 256, :].rearrange("(h p) d -> p h d", p=128),
                osb[:],
            )

        # Interleave: projection matmul groups between attention blocks keep
        # PE busy while exp/mask/PV chains drain on Act/Pool/DVE.
        emit_proj_qk(0)
        emit_proj_v(0)
        emit_proj_qk(1)
        emit_proj_v(1)
        emit_block(0)
        nb_next = 1
        for t in range(2, 8):
            emit_proj_qk(t)
            emit_block(nb_next)
            emit_proj_v(t)
            emit_block(nb_next + 1)
            nb_next += 2
        for n in range(nb_next, NB):
            emit_block(n)

    nc.compile()
    return nc


_prog_cache = {}


def _get_program(has_bias, has_kmask):
    key = (has_bias, has_kmask)
    if key not in _prog_cache:
        _prog_cache[key] = build_program(has_bias, has_kmask)
    return _prog_cache[key]


def _band_masks():
    """[L|L|U|U] multiplicative masks, [128, 512] bf16.

    L[r, j] = (j <= r) masks [mL1-tri | mL2]; U[r, j] = (j >= r) masks
    [mR2 | mR1-tri].
    """
    r = np.arange(128)[:, None]
    j = np.arange(128)[None, :]
    L = (j <= r).astype(np.float32)
    U = (j >= r).astype(np.float32)
    return np.concatenate([L, L, U, U], axis=1).astype(ml_dtypes.bfloat16)


def kernel(hidden_states, attention_mask, Wq, bq, Wk, bk, Wv, bv, _res=[None]):
    hidden_states = np.asarray(hidden_states, np.float32)
    attention_mask = np.asarray(attention_mask, np.float32)
    Wq, Wk, Wv = (np.asarray(w, np.float32) for w in (Wq, Wk, Wv))
    bq, bk, bv = (np.asarray(b_, np.float32) for b_ in (bq, bk, bv))

    scale = 1.0 / np.sqrt(DH)
    has_bias = bool(np.any(bq) or np.any(bk) or np.any(bv))
    has_kmask = bool(np.any(attention_mask < 0))

    hsT = [
        np.ascontiguousarray(hidden_states[b].T).astype(ml_dtypes.bfloat16)
        for b in range(B)
    ]
    masks = _band_masks()
    masked = attention_mask < 0  # [B, S]

    in_maps = []
    for core in range(N_CORES):
        b, h0 = core // 4, (core % 4) * HPC
        sl = slice(h0 * DH, (h0 + HPC) * DH)
        wq = Wq[:, sl] * scale
        wk = Wk[:, sl]
        wqkv = np.concatenate(
            [wq[:, 0:128], wk[:, 0:128], wq[:, 128:192], wk[:, 128:192],
             Wv[:, sl]],
            axis=1,
        ).astype(ml_dtypes.bfloat16)
        m = {
            "hsT": hsT[b],
            "wqkv": np.ascontiguousarray(wqkv),
            "masks": masks,
        }
        if has_bias:
            bq_s = bq[sl] * scale
            bk_s = bk[sl]
            m["bqkv"] = np.concatenate(
                [bq_s[0:128], bk_s[0:128], bq_s[128:192], bk_s[128:192],
                 bv[sl]]
            ).reshape(1, 576).astype(ml_dtypes.bfloat16)
        if has_kmask:
            keep = (~masked[b]).astype(np.float32).reshape(NKC, 128).T
            m["kpad"] = np.ascontiguousarray(keep)
            m["qpad"] = np.ascontiguousarray(keep)
        in_maps.append(m)

    nc = _get_program(has_bias, has_kmask)
    res = run_bass_kernel_spmd(nc, in_maps, list(range(N_CORES)))
    _res[0] = res

    out = np.empty((B, S, D), np.float32)
    for core in range(N_CORES):
        b, h0 = core // 4, (core % 4) * HPC
        out[b, :, h0 * DH : (h0 + HPC) * DH] = res.results[core]["out"]
    return out


# revision 9
# speedup vs baseline: 1.5113x; 1.0468x over previous
"""Longformer-style windowed self-attention for TRN2, 8-core SPMD.

Sharding: 24 (batch, head) pairs -> 3 heads per core (core c gets batch c//4,
heads (c%4)*3 .. +3). Each core computes QKV projections for its head slice,
windowed attention (block 256, window +-256), and writes its [4096, 192]
output channel slice. Host gathers slices into the full [2, 4096, 768] output.

All matmul inputs are bf16 (inputs/weights converted on host). Scores are
computed transposed ([keys, queries]); probs (exp'd scores) become the
stationary operand of the PV matmul, which therefore produces output directly
in [queries, head_dim] layout with a ones-column carrying the softmax
denominator - no PE transposes needed. Band-mask multiplies run on GpSimd,
exp on the scalar engine, PSUM evacuation + normalize scaling on DVE.
"""

import sys

for _p in ("/opt/trn_rl_repo", "/opt/pypackages"):
    if _p not in sys.path:
        sys.path.append(_p)

import numpy as np
import ml_dtypes
from contextlib import ExitStack

import concourse.bass as bass
import concourse.bacc as bacc
import concourse.mybir as mybir
import concourse.tile as tile
from concourse.bass_utils import run_bass_kernel_spmd

F32 = mybir.dt.float32
BF16 = mybir.dt.bfloat16
EXP = mybir.ActivationFunctionType.Exp
MUL = mybir.AluOpType.mult

B, S, D = 2, 4096, 768
H, DH = 12, 64
W = 256                 # one-sided window / query block size
NB = S // W             # 16 query blocks
NKC = S // 128          # 32 key chunks of 128
HPC = 3                 # heads per core
N_CORES = 8


def block_layout(n):
    """Score-PSUM column layout for query block n.

    Returns (pieces, maskop, ncols). pieces = [(m, qlo, qhi, col)]: key chunk
    m's scores for local queries [qlo, qhi) live at psum cols [col, col+qhi-qlo).
    maskop = (dst_col, width, src_col) multiplies pt[:, dst:dst+width] by
    msk[:, src:src+width] (msk = [L|L|U|U]). 256-wide pieces sit at byte
    offsets that never straddle a 2KB PSUM bank.
    """
    if n == 0:
        pieces = [(0, 0, 256, 0), (1, 0, 256, 256),
                  (3, 128, 256, 512), (2, 0, 256, 640)]
        maskop = (512, 256, 256)  # [mR2 | mR1 tri] *= [U|U]
        ncols = 896
    elif n == NB - 1:
        m0 = 2 * n
        pieces = [(m0, 0, 256, 0), (m0 - 1, 0, 256, 256),
                  (m0 - 2, 0, 128, 512), (m0 + 1, 0, 256, 640)]
        maskop = (384, 256, 0)    # [mL1 tri | mL2] *= [L|L]
        ncols = 896
    else:
        pieces = [(2 * n - 1, 0, 256, 0), (2 * n - 2, 0, 128, 256),
                  (2 * n + 3, 128, 256, 384), (2 * n + 2, 0, 256, 512),
                  (2 * n, 0, 256, 768), (2 * n + 1, 0, 256, 1024)]
        maskop = (128, 512, 0)    # [mL1 tri | mL2 | mR2 | mR1 tri] *= [L|L|U|U]
        ncols = 1280
    return pieces, maskop, ncols


def pv_chunks(pieces, half):
    """(m, pt_col) for key chunks fully covering query half [128h, 128h+128)."""
    q0, q1 = 128 * half, 128 * half + 128
    return [(m, col + q0 - qlo) for (m, qlo, qhi, col) in pieces
            if qlo <= q0 and q1 <= qhi]


def build_program(has_bias, has_kmask):
    nc = bacc.Bacc("TRN2", target_bir_lowering=False, debug=False,
                   num_devices=N_CORES)
    hsT_d = nc.declare_dram_parameter("hsT", [D, S], BF16, isOutput=False)
    w_d = nc.declare_dram_parameter("wqkv", [D, 576], BF16, isOutput=False)
    msk_d = nc.declare_dram_parameter("masks", [128, 512], BF16, isOutput=False)
    if has_bias:
        bqkv_d = nc.declare_dram_parameter("bqkv", [1, 576], BF16, isOutput=False)
    if has_kmask:
        kpad_d = nc.declare_dram_parameter("kpad", [128, NKC], F32, isOutput=False)
        qpad_d = nc.declare_dram_parameter("qpad", [128, NKC], F32, isOutput=False)
    out_d = nc.declare_dram_parameter("out", [S, HPC * DH], F32, isOutput=True)

    with tile.TileContext(nc) as tc, ExitStack() as ctx:
        const_p = ctx.enter_context(tc.tile_pool(name="const", bufs=1))
        hst_p = ctx.enter_context(tc.tile_pool(name="hst", bufs=3))
        qkt_p = ctx.enter_context(tc.tile_pool(name="qkt", bufs=1))
        vall_p = ctx.enter_context(tc.tile_pool(name="vall", bufs=1))
        pt_p = ctx.enter_context(tc.tile_pool(name="pt", bufs=4))
        wk_p = ctx.enter_context(tc.tile_pool(name="wk", bufs=4))
        ps_p = ctx.enter_context(tc.tile_pool(name="ps", bufs=2, space="PSUM"))
        sm_p = ctx.enter_context(tc.tile_pool(name="sm", bufs=2, space="PSUM"))

        # ---- constants / weights ----
        wsb = const_p.tile([128, 6, 576], BF16)
        w_r = w_d[:].rearrange("(c p) n -> p c n", p=128)
        nc.sync.dma_start(wsb[:, :, 0:128], w_r[:, :, 0:128])

        hst_tiles = {}

        def dma_hst(t):
            hst = hst_p.tile([128, 6, 512], BF16)
            hst_tiles[t] = hst
            src = hsT_d[:].rearrange("(c p) s -> p c s", p=128)[
                :, :, 512 * t : 512 * t + 512
            ]
            if t == 0:  # split so the first projection group starts sooner
                nc.sync.dma_start(hst[:, 0:3, :], src[:, 0:3, :])
                nc.sync.dma_start(hst[:, 3:6, :], src[:, 3:6, :])
            else:
                nc.sync.dma_start(hst[:], src)

        dma_hst(0)
        nc.sync.dma_start(wsb[:, :, 128:576], w_r[:, :, 128:576])
        msk_sb = const_p.tile([128, 512], BF16)
        nc.sync.dma_start(msk_sb[:], msk_d[:, :])
        dma_hst(1)
        if has_bias:
            bqkv_sb = const_p.tile([1, 576], BF16)
            nc.sync.dma_start(bqkv_sb[:], bqkv_d[:, :])
            ones_sb = const_p.tile([1, 512], BF16)
            nc.vector.memset(ones_sb[:], 1.0)
        if has_kmask:
            kpad_sb = const_p.tile([128, NKC], F32)
            nc.sync.dma_start(kpad_sb[:], kpad_d[:, :])
            qpad_sb = const_p.tile([128, NKC], F32)
            nc.sync.dma_start(qpad_sb[:], qpad_d[:, :])

        # qT/kT for head pair (A,B): A on partitions 0:64, B on 64:128
        qt_ab = qkt_p.tile([128, S], BF16)
        kt_ab = qkt_p.tile([128, S], BF16)
        # solo head C: base-0 tiles
        qt_c = qkt_p.tile([64, S], BF16)
        kt_c = qkt_p.tile([64, S], BF16)
        # v in [key, dh] layout: [128, key-chunk, (vA|1|vB|1|vC|1)]
        vall = vall_p.tile([128, NKC, 195], BF16)
        ones_cols = vall[:].rearrange("p m (h x) -> p m h x", h=3)[:, :, :, 64:65]
        nc.vector.memset(ones_cols, 1.0)

        def emit_proj_qk(t):
            s0 = 512 * t
            if t + 1 < 8:
                dma_hst(t + 1)
            hst = hst_tiles[t]
            for j in range(3):
                pp = sm_p.tile([128, 512], F32, space="PSUM", tag="sm")
                for c in range(6):
                    nc.tensor.matmul(
                        pp[:],
                        (wsb[:, c, 128 * j : 128 * j + 128]),
                        (hst[:, c, :]),
                        start=(c == 0),
                        stop=(c == 5 and not has_bias),
                    )
                if has_bias:
                    nc.tensor.matmul(
                        pp[:],
                        (bqkv_sb[0:1, 128 * j : 128 * j + 128]),
                        (ones_sb[0:1, :]),
                        start=False,
                        stop=True,
                    )
                if j == 0:
                    nc.vector.tensor_copy(qt_ab[:, s0 : s0 + 512], pp[:])
                elif j == 1:
                    nc.vector.tensor_copy(kt_ab[:, s0 : s0 + 512], pp[:])
                else:
                    nc.vector.tensor_copy(qt_c[:, s0 : s0 + 512], pp[0:64, :])
                    kcs = wk_p.tile([128, 512], BF16, name="kcs")
                    nc.vector.tensor_copy(kcs[64:128, :], pp[64:128, :])
                    nc.sync.dma_start(kt_c[:, s0 : s0 + 512], kcs[64:128, :])

        def emit_proj_v(t):
            hst = hst_tiles.pop(t)
            for mm0 in (0, 2):
                m = 4 * t + mm0
                pv = sm_p.tile([128, 512], F32, space="PSUM", tag="sm")
                for half, mm in enumerate((mm0, mm0 + 1)):
                    for c in range(6):
                        nc.tensor.matmul(
                            pv[:, 256 * half : 256 * half + 192],
                            (hst[:, c, 128 * mm : 128 * mm + 128]),
                            (wsb[:, c, 384:576]),
                            start=(c == 0),
                            stop=(c == 5 and not has_bias),
                        )
                    if has_bias:
                        nc.tensor.matmul(
                            pv[:, 256 * half : 256 * half + 192],
                            (ones_sb[0:1, 0:128]),
                            (bqkv_sb[0:1, 384:576]),
                            start=False,
                            stop=True,
                        )
                dst = vall[:, m : m + 2, :].rearrange(
                    "p m (h x) -> p m h x", h=3
                )[:, :, :, 0:64]
                src = pv[:].rearrange("p (m x) -> p m x", m=2)[
                    :, :, 0:192
                ].rearrange("p m (h x) -> p m h x", h=3)
                nc.vector.tensor_copy(dst, src)

        HEADS = (
            (lambda: kt_ab[0:64, :], lambda: qt_ab[0:64, :]),
            (lambda: kt_ab[64:128, :], lambda: qt_ab[64:128, :]),
            (lambda: kt_c[:, :], lambda: qt_c[:, :]),
        )

        def emit_block(n):
            pieces, maskop, ncols = block_layout(n)
            q0 = 256 * n
            pts = []
            for h, (ktf, qtf) in enumerate(HEADS):
                kt, qt = ktf(), qtf()
                ps = ps_p.tile([128, 1280], F32, space="PSUM", tag="ps")
                for m, qlo, qhi, col in pieces:
                    nc.tensor.matmul(
                        ps[:, col : col + qhi - qlo],
                        (kt[:, 128 * m : 128 * m + 128]),
                        (qt[:, q0 + qlo : q0 + qhi]),
                        start=True,
                        stop=True,
                    )
                pt = pt_p.tile([128, 1280], BF16, tag="pt")
                pts.append(pt)
                nc.scalar.activation(pt[:, 0:ncols], ps[:, 0:ncols], EXP)
                dcol, width, scol = maskop
                nc.vector.scalar_tensor_tensor(
                    pt[:, dcol : dcol + width],
                    pt[:, dcol : dcol + width],
                    1.0,
                    msk_sb[:, scol : scol + width],
                    MUL,
                    MUL,
                )
                if has_kmask:
                    for m, qlo, qhi, col in pieces:
                        nc.vector.tensor_scalar_mul(
                            pt[:, col : col + qhi - qlo],
                            pt[:, col : col + qhi - qlo],
                            kpad_sb[:, m : m + 1],
                        )

            # PV: out[q, dh] = pt(chunk).T @ [v|1]; col 64 of each head's rhs
            # slice is the ones column carrying the softmax denominator.
            outp = sm_p.tile([128, 512], F32, space="PSUM", tag="sm")
            dcol, width, _ = maskop
            for h, pt in enumerate(pts):
                for half in (0, 1):
                    chunks = pv_chunks(pieces, half)
                    # unmasked chunks first: their matmuls only depend on the
                    # exp, so PV starts while the mask op is still running
                    chunks.sort(
                        key=lambda mp: not (
                            mp[1] + 128 <= dcol or mp[1] >= dcol + width
                        )
                    )
                    for ci, (m, pcol) in enumerate(chunks):
                        nc.tensor.matmul(
                            outp[:, 256 * half + 65 * h : 256 * half + 65 * h + 65],
                            (pt[:, pcol : pcol + 128]),
                            (vall[:, m, 65 * h : 65 * h + 65]),
                            start=(ci == 0),
                            stop=(ci == len(chunks) - 1),
                        )

            rec = wk_p.tile([128, 8], F32, name="rec")
            osb = wk_p.tile([128, 2, 192], F32, name="osb")
            for half in (0, 1):
                dcols = outp[:, 256 * half : 256 * half + 195].rearrange(
                    "p (i x) -> p i x", x=65
                )[:, :, 64:65]
                nc.vector.reciprocal(
                    rec[:, 4 * half : 4 * half + 3].rearrange(
                        "p (i x) -> p i x", x=1
                    ),
                    dcols,
                )
                for h in range(3):
                    nc.vector.tensor_scalar_mul(
                        osb[:, half, 64 * h : 64 * h + 64],
                        outp[:, 256 * half + 65 * h : 256 * half + 65 * h + 64],
                        rec[:, 4 * half + h : 4 * half + h + 1],
                    )
                if has_kmask:
                    nc.vector.tensor_scalar_mul(
                        osb[:, half, :],
                        osb[:, half, :],
                        qpad_sb[:, 2 * n + half : 2 * n + half + 1],
                    )
            if n == NB - 1:  # split the final DMA so half0 drains early
                for half in (0, 1):
                    nc.sync.dma_start(
                        out_d[q0 + 128 * half : q0 + 128 * half + 128, :],
                        osb[:, half, :],
                    )
            else:
                nc.sync.dma_start(
                    out_d[q0 : q0 + 256, :].rearrange(
                        "(h p) d -> p h d", p=128
                    ),
                    osb[:],
                )

        # Interleave: projection matmul groups between attention blocks keep
        # PE busy while exp/mask/PV chains drain on Act/Pool/DVE.
        emit_proj_qk(0)
        emit_proj_v(0)
        emit_proj_qk(1)
        emit_proj_v(1)
        emit_block(0)
        nb_next = 1
        for t in range(2, 8):
            emit_proj_qk(t)
            emit_block(nb_next)
            emit_proj_v(t)
            emit_block(nb_next + 1)
            nb_next += 2
        for n in range(nb_next, NB):
            emit_block(n)

    nc.compile()
    return nc


_prog_cache = {}


def _get_program(has_bias, has_kmask):
    key = (has_bias, has_kmask)
    if key not in _prog_cache:
        _prog_cache[key] = build_program(has_bias, has_kmask)
    return _prog_cache[key]


def _band_masks():
    """[L|L|U|U] multiplicative masks, [128, 512] bf16.

    L[r, j] = (j <= r) masks [mL1-tri | mL2]; U[r, j] = (j >= r) masks
    [mR2 | mR1-tri].
    """
    r = np.arange(128)[:, None]
    j = np.arange(128)[None, :]
    L = (j <= r).astype(np.float32)
    U = (j >= r).astype(np.float32)
    return np.concatenate([L, L, U, U], axis=1).astype(ml_dtypes.bfloat16)


def kernel(hidden_states, attention_mask, Wq, bq, Wk, bk, Wv, bv, _res=[None]):
    hidden_states = np.asarray(hidden_states, np.float32)
    attention_mask = np.asarray(attention_mask, np.float32)
    Wq, Wk, Wv = (np.asarray(w, np.float32) for w in (Wq, Wk, Wv))
    bq, bk, bv = (np.asarray(b_, np.float32) for b_ in (bq, bk, bv))

    scale = 1.0 / np.sqrt(DH)
    has_bias = bool(np.any(bq) or np.any(bk) or np.any(bv))
    has_kmask = bool(np.any(attention_mask < 0))

    hsT = [
        np.ascontiguousarray(hidden_states[b].T).astype(ml_dtypes.bfloat16)
        for b in range(B)
    ]
    masks = _band_masks()
    masked = attention_mask < 0  # [B, S]

    in_maps = []
    for core in range(N_CORES):
        b, h0 = core // 4, (core % 4) * HPC
        sl = slice(h0 * DH, (h0 + HPC) * DH)
        wq = Wq[:, sl] * scale
        wk = Wk[:, sl]
        wqkv = np.concatenate(
            [wq[:, 0:128], wk[:, 0:128], wq[:, 128:192], wk[:, 128:192],
             Wv[:, sl]],
            axis=1,
        ).astype(ml_dtypes.bfloat16)
        m = {
            "hsT": hsT[b],
            "wqkv": np.ascontiguousarray(wqkv),
            "masks": masks,
        }
        if has_bias:
            bq_s = bq[sl] * scale
            bk_s = bk[sl]
            m["bqkv"] = np.concatenate(
                [bq_s[0:128], bk_s[0:128], bq_s[128:192], bk_s[128:192],
                 bv[sl]]
            ).reshape(1, 576).astype(ml_dtypes.bfloat16)
        if has_kmask:
            keep = (~masked[b]).astype(np.float32).reshape(NKC, 128).T
            m["kpad"] = np.ascontiguousarray(keep)
            m["qpad"] = np.ascontiguousarray(keep)
        in_maps.append(m)

    nc = _get_program(has_bias, has_kmask)
    res = run_bass_kernel_spmd(nc, in_maps, list(range(N_CORES)))
    _res[0] = res

    out = np.empty((B, S, D), np.float32)
    for core in range(N_CORES):
        b, h0 = core // 4, (core % 4) * HPC
        out[b, :, h0 * DH : (h0 + HPC) * DH] = res.results[core]["out"]
    return out
